# revision 3
# baseline (speedup 1.0000x reference)
"""Trainium2 Bass kernel for nn_GCK3x3Layer: 3x3 VALID conv, 256->256 ch.

result = kernelsL @ im2col_3x3(input); input (1,256,258,258) f32,
kernelsL (256, 2304) f32 -> output (1, 256, 256, 256) f32.

Strategy: spatial-parallel across 8 NeuronCores (each core owns a 32-row
output strip, input rows shared with a 2-row halo), Winograd F(2x2,3x3)
in bf16 on each core:

  y = A^T [ (G g G^T) . (B^T d B) ] A   per 4x4 input tile (stride 2)

batched over channels as 16 per-tap GEMMs [oc,ic]x[ic,tiles] on the
tensor engine (PE columns: 131072/core vs 294912 for direct conv).
The input transform (DVE), PSUM->SBUF drain (ACT), M@A (GPSIMD) and
A^T@ (DVE) stages run pipelined one chunk (4 tile-rows) ahead of the
GEMMs.  Host pre-casts inputs to bf16 and pre-computes U = G g G^T
(full f32 accuracy kept in PSUM accumulation; rel err ~1e-2).
"""

import os
import sys
from contextlib import ExitStack

import numpy as np

for _p in (
    "/root/.axon_site",
    "/root/.axon_site/_ro/trn_rl_repo",
    "/root/.axon_site/_ro/pypackages",
    "/opt/trn_rl_repo",
):
    if os.path.isdir(_p) and _p not in sys.path:
        sys.path.append(_p)

import concourse.bass as bass  # noqa: E402,F401
import concourse.tile as tile  # noqa: E402
from concourse import bacc, mybir  # noqa: E402
from concourse.bass_utils import run_bass_kernel_spmd  # noqa: E402

IN_C = 256
OUT_C = 256
H = 258
W = 258
H_OUT = H - 2  # 256
W_OUT = W - 2  # 256
NCORES = 8
ROWS_PER_CORE = H_OUT // NCORES  # 32
IN_ROWS = ROWS_PER_CORE + 2  # 34
P = 128
ICB = IN_C // P  # 2 input-channel blocks
OCB = OUT_C // P  # 2 output-channel blocks

F32 = mybir.dt.float32
BF16 = mybir.dt.bfloat16
NCHUNK = 4        # chunks per pass
CTR = 4           # tile-rows per chunk
NT = CTR * P      # 512 tiles per chunk


def build(loop_repeat=1, repeat=1, staggered=False, engines=None,
          use_copy=True, warm=12):
    """Build + compile the per-core Winograd Bass program.

    engines: dict stage -> tuple of engine names cycled per instruction:
      row/col: input transform, ma: M@A combine, at: A^T@ combine,
      copy: PSUM->SBUF bf16 drain ("scalar" = ACT engine).
    loop_repeat/repeat: hardware For_i reps / unrolled reps (dev timing).
    """
    eng_spec = {
        "row": ("vector",), "col": ("vector",),
        "ma": ("gpsimd",), "at": ("vector",), "copy": ("scalar",),
    }
    if engines:
        eng_spec.update(engines)

    nc = bacc.Bacc(
        "TRN2", target_bir_lowering=False, debug=False, num_devices=NCORES
    )
    x = nc.dram_tensor("x", [IN_C, IN_ROWS * W], BF16, kind="ExternalInput")
    u = nc.dram_tensor("u", [IN_C, 16 * OUT_C], BF16, kind="ExternalInput")
    y = nc.dram_tensor(
        "y", [OUT_C, ROWS_PER_CORE * W_OUT], F32, kind="ExternalOutput"
    )

    xv = x.rearrange("(b p) (r c) -> p b r c", p=P, c=W)
    uv = u.rearrange("(b p) (t m) -> p b t m", p=P, t=16)

    ENG = {"vector": nc.vector, "gpsimd": nc.gpsimd, "scalar": nc.scalar}
    counters = {k: 0 for k in eng_spec}

    def eng(stage):
        names = eng_spec[stage]
        e = ENG[names[counters[stage] % len(names)]]
        counters[stage] += 1
        return e

    with tile.TileContext(nc) as tc:
        with ExitStack() as ctx:
            xpool = ctx.enter_context(tc.tile_pool(name="xp", bufs=2))
            upool = ctx.enter_context(tc.tile_pool(name="up", bufs=1))
            rpool = ctx.enter_context(tc.tile_pool(name="rp", bufs=1))
            vpool = ctx.enter_context(tc.tile_pool(name="vp", bufs=2))
            pspool = ctx.enter_context(
                tc.tile_pool(name="ps", bufs=2, space="PSUM")
            )
            mpool = ctx.enter_context(tc.tile_pool(name="mp", bufs=3))
            p2pool = ctx.enter_context(tc.tile_pool(name="p2", bufs=2))
            ypool = ctx.enter_context(tc.tile_pool(name="yp", bufs=2))
            tpool = ctx.enter_context(tc.tile_pool(name="tp", bufs=2))

            # HAM warmup (PE clock ramp) while the weight DMA lands.
            wtile = upool.tile([P, P], F32, name="warm")
            nc.gpsimd.memset(wtile[:], 0.0)
            wps = pspool.tile([P, 4, NT], F32, name="wps", tag="wps")
            for _ in range(warm):
                nc.tensor.matmul(
                    wps[:, 0, 0:P], wtile[:], wtile[:],
                    start=True, stop=True, skip_group_check=True,
                )

            u_sb = upool.tile([P, ICB, 16, OUT_C], BF16)
            for icb in range(ICB):
                nc.sync.dma_start(u_sb[:, icb, :, 0:P], uv[:, icb, :, 0:P])
                nc.sync.dma_start(
                    u_sb[:, icb, :, P:OUT_C], uv[:, icb, :, P:OUT_C]
                )

            XH = 18  # rows per x half-buffer (2 chunks + halo)

            def _transforms(c, xh):
                """Row+col input transform for chunk c -> V tile."""
                x_q = xh[c // 2]
                cl = c % 2  # chunk-local within the half

                rt = rpool.tile([P, ICB, 4, 4, W], BF16, name="rt")

                def xa(a):
                    lo = 4 * cl + a // 2
                    return x_q[:, :, lo:lo + 4, a % 2, :]

                eng("row").tensor_sub(rt[:, :, 0], xa(0), xa(2))
                eng("row").tensor_add(rt[:, :, 1], xa(1), xa(2))
                eng("row").tensor_sub(rt[:, :, 2], xa(2), xa(1))
                eng("row").tensor_sub(rt[:, :, 3], xa(1), xa(3))

                # merged view: (t_r a) fused, w split into (j parity)
                r_q = rt.rearrange("p i t a (j q) -> p i (t a) j q", q=2)
                v = vpool.tile([P, ICB, 4, 16, P], BF16, name="v")

                def rb(b):
                    return r_q[:, :, :, b // 2:b // 2 + P, b % 2]

                eng("col").tensor_sub(v[:, :, 0], rb(0), rb(2))
                eng("col").tensor_add(v[:, :, 1], rb(1), rb(2))
                eng("col").tensor_sub(v[:, :, 2], rb(2), rb(1))
                eng("col").tensor_sub(v[:, :, 3], rb(1), rb(3))
                return v

            def _gemm_tail(c, v):
                """16 tap-GEMMs + M@A + A^T@ + output DMA for chunk c."""
                for ocb in range(OCB):
                    # p2 layout: s LAST so the at-stage can merge s
                    p2 = p2pool.tile([P, 4, CTR, P, 2], BF16, name="p2")
                    for gp in range(2):  # pairs of t_r groups
                        m = mpool.tile([P, 2, 4, CTR, P], BF16, name="m")
                        for gi in range(2):
                            t_r = 2 * gp + gi
                            ps = pspool.tile([P, 4, NT], F32, tag="wps")
                            for t_c in range(4):
                                t = 4 * t_r + t_c
                                for icb in range(ICB):
                                    nc.tensor.matmul(
                                        ps[:, t_c, :],
                                        u_sb[:, icb, t,
                                             ocb * P:(ocb + 1) * P],
                                        v[:, icb, t_c,
                                          t_r * 4:(t_r + 1) * 4, :],
                                        start=(icb == 0),
                                        stop=(icb == ICB - 1),
                                    )
                            psv = ps.rearrange("p t (a j) -> p t a j", j=P)
                            ce = eng("copy")
                            if ce is nc.scalar:
                                ce.activation(
                                    m[:, gi], psv[:],
                                    mybir.ActivationFunctionType.Copy,
                                )
                            else:
                                ce.tensor_copy(m[:, gi], psv[:])
                        # M @ A over the pair (s=0: M0+M1+M2, s=1: M1-M2-M3)
                        e1, e2 = eng("ma"), eng("ma")
                        tmp = tpool.tile([P, 2, 2, CTR, P], BF16, name="mt")
                        p2s = p2[:, 2 * gp:2 * gp + 2]
                        e1.tensor_add(tmp[:, 0], m[:, :, 0], m[:, :, 1])
                        e1.tensor_add(
                            p2s[:, :, :, :, 0], tmp[:, 0], m[:, :, 2])
                        e2.tensor_sub(tmp[:, 1], m[:, :, 1], m[:, :, 2])
                        e2.tensor_sub(
                            p2s[:, :, :, :, 1], tmp[:, 1], m[:, :, 3])
                    # A^T stage, s merged: operands [a, j, s]
                    yst = ypool.tile([P, CTR, 2, P, 2], F32, name="yst")
                    t2 = tpool.tile([P, 2, CTR, P, 2], BF16, name="at")
                    ea, eb = eng("at"), eng("at")
                    ea.tensor_add(t2[:, 0], p2[:, 0], p2[:, 1])
                    ea.tensor_add(yst[:, :, 0], t2[:, 0], p2[:, 2])
                    eb.tensor_sub(t2[:, 1], p2[:, 1], p2[:, 2])
                    eb.tensor_sub(yst[:, :, 1], t2[:, 1], p2[:, 3])
                    nc.sync.dma_start(
                        y[
                            ocb * P:(ocb + 1) * P,
                            c * 8 * W_OUT:(c + 1) * 8 * W_OUT,
                        ],
                        yst.rearrange("p a r j s -> p (a r j s)"),
                    )

            def _one_pass():
                xh = []
                for h in range(2):
                    t = xpool.tile([P, ICB, XH, W], BF16, name="xh")
                    r0 = 16 * h
                    for b in range(ICB):
                        nc.sync.dma_start(
                            t[:, b, :, :], xv[:, b, r0:r0 + XH, :]
                        )
                    xh.append(t.rearrange("p i (rr q) w -> p i rr q w", q=2))

                # software pipeline: transforms run one chunk ahead of GEMM
                v_cur = _transforms(0, xh)
                for c in range(NCHUNK):
                    v_next = _transforms(c + 1, xh) if c + 1 < NCHUNK else None
                    _gemm_tail(c, v_cur)
                    v_cur = v_next

            if loop_repeat > 1:
                for_kwargs = {}
                if staggered:
                    for_kwargs["staggered_reset"] = True
                with tc.For_i(0, loop_repeat, 1, **for_kwargs):
                    for _rep in range(repeat):
                        _one_pass()
            else:
                for _rep in range(repeat):
                    _one_pass()
    nc.compile()
    return nc


def make_in_maps(input, kernelsL):
    import ml_dtypes

    inp = np.asarray(input, dtype=np.float32).reshape(IN_C, H, W)
    w = np.asarray(kernelsL, dtype=np.float32).reshape(OUT_C, IN_C, 3, 3)
    G = np.array(
        [[1, 0, 0], [0.5, 0.5, 0.5], [0.5, -0.5, 0.5], [0, 0, 1]], np.float32
    )
    # U[oc, ic, 4, 4] = G @ g @ G^T ; dram layout [ic, t*256 + oc]
    U = np.einsum("ta,oiab,ub->oitu", G, w, G, optimize=True)
    Ud = np.ascontiguousarray(
        U.reshape(OUT_C, IN_C, 16).transpose(1, 2, 0).reshape(IN_C, 16 * OUT_C)
    ).astype(ml_dtypes.bfloat16)
    in_maps = []
    for c in range(NCORES):
        r0 = c * ROWS_PER_CORE
        strip = (
            np.ascontiguousarray(inp[:, r0:r0 + IN_ROWS, :])
            .reshape(IN_C, IN_ROWS * W)
            .astype(ml_dtypes.bfloat16)
        )
        in_maps.append({"x": strip, "u": Ud})
    return in_maps


def assemble(results):
    out = np.empty((OUT_C, H_OUT, W_OUT), dtype=np.float32)
    for c in range(NCORES):
        out[:, c * ROWS_PER_CORE:(c + 1) * ROWS_PER_CORE, :] = results[c][
            "y"
        ].reshape(OUT_C, ROWS_PER_CORE, W_OUT)
    return out.reshape(1, OUT_C, H_OUT, W_OUT)


_NC_CACHE = {}


def _get_nc():
    if "nc" not in _NC_CACHE:
        _NC_CACHE["nc"] = build()
    return _NC_CACHE["nc"]


def kernel(input, kernelsL):
    in_maps = make_in_maps(input, kernelsL)
    nc = _get_nc()
    res = run_bass_kernel_spmd(nc, in_maps, core_ids=list(range(NCORES)))
    return assemble(res.results)
